# revision 3
# baseline (speedup 1.0000x reference)
"""Trainium2 Bass kernel: multi-head attention (B=4, T=2048, D=2048, H=16).

Sharding: 8 cores = 4 batches x 2 head-groups (tensor-parallel heads, data-
parallel batch). Each core handles one batch and 8 heads (f-slice of 1024
columns of the QKV projections / rows of the out-projection). Host sums the
two partial out-projection results per batch and adds the output bias.

Per-core pipeline:
  Phase A: QKV projections in fp8e4 DoubleRow mode (4x bf16 rate) with
           3-term error compensation: x ~ xh+xl, 32*W ~ wh+wl (both split
           host-side; the x32 scaling keeps wl out of fp8-subnormal range),
           y = (xh@wh + xl@wh + xh@wl)/32.  qT[f,t], kT[f,t] are
           weight-stationary; v[t,f] is x-stationary.  24 DR matmuls per
           512-wide PSUM pair accumulate K=2048 in pairs of 128-chunks.
  Phase B: per (head, q-half) iteration i: S^T[k,q] = kT.T @ qT on PE;
           exp(scale*S^T) on ScalarE -> P^T bf16; PV with ones-augmented V
           gives [q, dh | rowsum]; normalize rows (DVE reciprocal + scale),
           PE-transpose y -> yT[f,t] (+bv fused, softmax rows sum to 1).
           Iterations are software-pipelined: the PV/normalize work of
           iteration i-1 is interleaved between the 16 score chunks of
           iteration i so PE and ScalarE both stay busy.
  Phase C: out^T[d,t] = WoT.T @ yT accumulated over f-chunks (bf16),
           written bf16; host transposes back, sums the two head-group
           partials in f32 and adds the output bias.  Wo tiles are
           prefetched during phase B.
"""

import sys

if "/opt/trn_rl_repo" not in sys.path:
    sys.path.insert(0, "/opt/trn_rl_repo")

import numpy as np
import ml_dtypes

D = 2048          # d_model
T = 2048          # sequence length
B = 4             # batch
H = 16            # total heads
DH = 128          # head dim
GROUPS = 2        # head groups (tensor-parallel factor per batch)
HG = H // GROUPS  # heads per core = 8
F = HG * DH       # per-core projection width = 1024
P = 128
DC = D // P       # 16 contraction chunks
TC = T // P       # 16 t chunks
NCORES = 8
SCALE = float(1.0 / np.sqrt(DH))
WSCALE = 32.0     # pre-scale on W before fp8 split (avoids fp8 subnormals)

_PROGRAM = None


def _build_program():
    import concourse.bass as bass
    import concourse.tile as tile
    from concourse import bacc, mybir
    from concourse.bass import ts, ds
    from concourse.masks import make_identity

    bf16 = mybir.dt.bfloat16
    fp8 = mybir.dt.float8e4
    f32 = mybir.dt.float32
    DR = mybir.MatmulPerfMode.DoubleRow

    nc = bacc.Bacc("TRN2", target_bir_lowering=False, debug=False,
                   num_devices=NCORES)

    xh_d = nc.dram_tensor("xh", [DC, P, T], fp8, kind="ExternalInput")
    xl_d = nc.dram_tensor("xl", [DC, P, T], fp8, kind="ExternalInput")
    w_d = {}
    for kind in ("q", "k", "v"):
        w_d[kind] = (
            nc.dram_tensor(f"w{kind}h", [DC, P, F], fp8, kind="ExternalInput"),
            nc.dram_tensor(f"w{kind}l", [DC, P, F], fp8, kind="ExternalInput"),
        )
    wo_d = nc.dram_tensor("wo", [HG, P, D], bf16, kind="ExternalInput")
    bq_d = nc.dram_tensor("bq", [P, HG], f32, kind="ExternalInput")
    bk_d = nc.dram_tensor("bk", [P, HG], f32, kind="ExternalInput")
    bv_d = nc.dram_tensor("bv", [P, HG], f32, kind="ExternalInput")
    out_d = nc.dram_tensor("out", [DC, P, T], bf16, kind="ExternalOutput")

    Exp = mybir.ActivationFunctionType.Exp
    Identity = mybir.ActivationFunctionType.Identity
    INV_WS = float(1.0 / WSCALE)

    with tile.TileContext(nc) as tc:
        from contextlib import ExitStack
        with ExitStack() as ctx:
            # ---- persistent pools (allocated first, live whole kernel) ----
            const = ctx.enter_context(tc.tile_pool(name="const", bufs=1))
            qkt = ctx.enter_context(tc.tile_pool(name="qkt", bufs=1))
            vpool = ctx.enter_context(tc.tile_pool(name="vpool", bufs=1))
            ytp = ctx.enter_context(tc.tile_pool(name="ytp", bufs=1))

            ident = const.tile([P, P], bf16, tag="ident")
            make_identity(nc, ident)
            zero_b = const.tile([P, 1], f32, tag="zerob")
            nc.vector.memset(zero_b[:], 0.0)
            bq_sb = const.tile([P, HG], f32, tag="bq")
            bk_sb = const.tile([P, HG], f32, tag="bk")
            bv_sb = const.tile([P, HG], f32, tag="bv")
            nc.sync.dma_start(bq_sb[:], bq_d[:])
            nc.sync.dma_start(bk_sb[:], bk_d[:])
            nc.sync.dma_start(bv_sb[:], bv_d[:])

            qT = [qkt.tile([P, T], bf16, tag=f"qT{h}", name=f"qT{h}")
                  for h in range(HG)]
            kT = [qkt.tile([P, T], bf16, tag=f"kT{h}", name=f"kT{h}")
                  for h in range(HG)]
            v_sb = vpool.tile([P, TC, HG, DH + 1], bf16, tag="v")
            yT = ytp.tile([P, HG, T], bf16, tag="yT")

            # force early allocation of persistent pools (first-use order)
            nc.vector.memset(qT[0][:, 0:1], 0.0)
            nc.vector.memset(v_sb[:, :, :, DH:DH + 1], 1.0)
            nc.vector.memset(yT[:, 0, 0:1], 0.0)

            # ---------------- Phase A: projections (fp8 DoubleRow) --------
            with tc.tile_pool(name="wpool", bufs=17) as wpool, \
                 tc.tile_pool(name="xpool", bufs=2) as xpool, \
                 tc.tile_pool(name="ps_proj", bufs=4, space="PSUM") as ps_proj:
                for kind, bias_sb in (("q", bq_sb), ("k", bk_sb),
                                      ("v", None)):
                    wh_d, wl_d = w_d[kind]
                    wsb = {}
                    for half, wd in (("h", wh_d), ("l", wl_d)):
                        tiles = []
                        for pc in range(8):
                            wt = wpool.tile([P, 2, F], fp8, tag="wch",
                                            name=f"w{kind}{half}{pc}")
                            nc.sync.dma_start(
                                wt[:],
                                wd[ds(2 * pc, 2)].rearrange("c p f -> p c f"))
                            tiles.append(wt)
                        wsb[half] = tiles
                    for tcb in range(4):  # t-blocks of 512
                        xht = xpool.tile([P, DC, 512], fp8, tag="xh",
                                         name=f"xh_{kind}{tcb}")
                        xlt = xpool.tile([P, DC, 512], fp8, tag="xl",
                                         name=f"xl_{kind}{tcb}")
                        for src_d, dst in ((xh_d, xht), (xl_d, xlt)):
                            src = src_d[:, :, ds(tcb * 512, 512)].rearrange(
                                "c p t -> p c t")
                            for dg in range(DC):
                                nc.sync.dma_start(dst[:, ds(dg, 1)],
                                                  src[:, ds(dg, 1)])
                        terms = ((wsb["h"], xht), (wsb["l"], xht),
                                 (wsb["h"], xlt))
                        if kind != "v":
                            dst = qT if kind == "q" else kT
                            for h in range(HG):
                                psA = ps_proj.tile([P, 256], f32, tag="ps",
                                                   name=f"psA_{kind}{tcb}{h}")
                                psB = ps_proj.tile([P, 256], f32, tag="ps",
                                                   name=f"psB_{kind}{tcb}{h}")
                                n = 0
                                for wset, xt in terms:
                                    for pc in range(8):
                                        lhsT = wset[pc][:, :, ds(h * DH, DH)]
                                        st_ = (n == 0)
                                        sp_ = (n == 23)
                                        nc.tensor.matmul(
                                            psA[:], lhsT,
                                            xt[:, ds(2 * pc, 2), 0:256],
                                            start=st_, stop=sp_, perf_mode=DR)
                                        nc.tensor.matmul(
                                            psB[:], lhsT,
                                            xt[:, ds(2 * pc, 2), 256:512],
                                            start=st_, stop=sp_, perf_mode=DR)
                                        n += 1
                                nc.scalar.activation(
                                    dst[h][:, ds(tcb * 512, 256)], psA[:],
                                    Identity, bias=bias_sb[:, ds(h, 1)],
                                    scale=INV_WS)
                                nc.scalar.activation(
                                    dst[h][:, ds(tcb * 512 + 256, 256)],
                                    psB[:], Identity,
                                    bias=bias_sb[:, ds(h, 1)], scale=INV_WS)
                        else:
                            for tsub in range(4):
                                tc_ = tcb * 4 + tsub
                                psl = ps_proj.tile([P, 512], f32, tag="ps",
                                                   name=f"psl{tc_}")
                                psr = ps_proj.tile([P, 512], f32, tag="ps",
                                                   name=f"psr{tc_}")
                                n = 0
                                for wset, xt in terms:
                                    for pc in range(8):
                                        lhs = xt[:, ds(2 * pc, 2),
                                                 ds(tsub * P, P)]
                                        st_ = (n == 0)
                                        sp_ = (n == 23)
                                        nc.tensor.matmul(
                                            psl[:], lhs,
                                            wset[pc][:, :, 0:512],
                                            start=st_, stop=sp_, perf_mode=DR)
                                        nc.tensor.matmul(
                                            psr[:], lhs,
                                            wset[pc][:, :, 512:1024],
                                            start=st_, stop=sp_, perf_mode=DR)
                                        n += 1
                                nc.scalar.activation(
                                    v_sb[:, tc_, 0:4, 0:DH],
                                    psl[:].rearrange("p (h d) -> p h d",
                                                     d=DH),
                                    Identity, bias=zero_b[:, :],
                                    scale=INV_WS)
                                nc.scalar.activation(
                                    v_sb[:, tc_, 4:8, 0:DH],
                                    psr[:].rearrange("p (h d) -> p h d",
                                                     d=DH),
                                    Identity, bias=zero_b[:, :],
                                    scale=INV_WS)

            # -------- Phases B+C share the wop/osb SBUF pools -------------
            with tc.tile_pool(name="wop", bufs=4) as wop, \
                 tc.tile_pool(name="osb", bufs=6) as osb:
                wo_tiles = []

                def wo_tile(dch):
                    wo_t = wop.tile([P, HG, P], bf16, tag="wo",
                                    name=f"wo{dch}")
                    nc.sync.dma_start(
                        wo_t[:],
                        wo_d[:, :, ds(dch * P, P)].rearrange(
                            "h p d -> p h d"))
                    return wo_t

                # ---------------- Phase B: attention ----------------
                with tc.tile_pool(name="ptpool", bufs=2) as ptpool, \
                     tc.tile_pool(name="ystage", bufs=4) as ystage, \
                     tc.tile_pool(name="rspool", bufs=4) as rspool, \
                     tc.tile_pool(name="ps_st", bufs=2,
                                  space="PSUM") as ps_st, \
                     tc.tile_pool(name="ps_pv", bufs=2,
                                  space="PSUM") as ps_pv, \
                     tc.tile_pool(name="ps_tr", bufs=2,
                                  space="PSUM") as ps_tr:
                    # prefetch first wo chunks during B (DMA idle here)
                    for dch in range(4):
                        wo_tiles.append(wo_tile(dch))

                    flat = [(h, half) for h in range(HG) for half in range(2)]
                    pt_prev = None
                    prv = None
                    for idx in range(len(flat) + 1):
                        cur = flat[idx] if idx < len(flat) else None
                        if cur is not None:
                            pt_cur = ptpool.tile([P, TC, T // 2], bf16,
                                                 tag="pt", name=f"pt{idx}")
                        pv = None
                        for kc in range(TC):
                            if cur is not None:
                                h, half = cur
                                q0 = half * (T // 2)
                                st = ps_st.tile([P, T // 2], f32, tag="st",
                                                name=f"st{idx}_{kc}")
                                for qc in range(2):
                                    nc.tensor.matmul(
                                        st[:, ds(qc * 512, 512)],
                                        kT[h][:, ds(kc * P, P)],
                                        qT[h][:, ds(q0 + qc * 512, 512)],
                                        start=True, stop=True)
                                nc.scalar.activation(pt_cur[:, kc], st[:],
                                                     Exp, bias=zero_b[:, :],
                                                     scale=SCALE)
                            if prv is not None:
                                ph, phalf = prv
                                pq0 = phalf * (T // 2)
                                qs, sub = divmod(kc, 2)
                                if sub == 0:
                                    pv = ps_pv.tile([P, DH + 1], f32,
                                                    tag="pv",
                                                    name=f"pv{idx}_{qs}")
                                for k2 in range(sub * 8, sub * 8 + 8):
                                    nc.tensor.matmul(
                                        pv[:],
                                        pt_prev[:, k2, ds(qs * P, P)],
                                        v_sb[:, k2, ph],
                                        start=(k2 == 0), stop=(k2 == TC - 1))
                                if sub == 1:
                                    rs = rspool.tile([P, 1], f32, tag="rs",
                                                     name=f"rs{idx}_{qs}")
                                    nc.vector.reciprocal(rs[:],
                                                         pv[:, DH:DH + 1])
                                    yst = ystage.tile([P, P], bf16,
                                                      tag="yst",
                                                      name=f"yst{idx}_{qs}")
                                    nc.vector.tensor_scalar_mul(
                                        yst[:], pv[:, 0:DH], rs[:])
                                    tr = ps_tr.tile([P, P], bf16, tag="tr",
                                                    name=f"tr{idx}_{qs}")
                                    nc.tensor.transpose(tr[:], yst[:],
                                                        ident[:])
                                    nc.vector.tensor_scalar_add(
                                        yT[:, ph, ds(pq0 + qs * P, P)],
                                        tr[:], bv_sb[:, ds(ph, 1)])
                        pt_prev = pt_cur
                        prv = cur

                # ---------------- Phase C: out-projection ----------------
                with tc.tile_pool(name="ps_o", bufs=8, space="PSUM") as ps_o:
                    for dch in range(DC):
                        wo_t = wo_tiles[dch] if dch < 4 else wo_tile(dch)
                        pso = [ps_o.tile([P, 512], f32, tag="pso",
                                         name=f"pso{dch}_{i}")
                               for i in range(4)]
                        for fc in range(HG):
                            for tcb in range(4):
                                nc.tensor.matmul(
                                    pso[tcb][:],
                                    wo_t[:, fc],
                                    yT[:, fc, ds(tcb * 512, 512)],
                                    start=(fc == 0), stop=(fc == HG - 1))
                        for tcb in range(4):
                            ot = osb.tile([P, 512], bf16, tag="ot",
                                          name=f"ot{dch}_{tcb}")
                            nc.vector.tensor_copy(ot[:], pso[tcb][:])
                            nc.sync.dma_start(
                                out_d[dch, :, ds(tcb * 512, 512)], ot[:])

    nc.compile()
    return nc


def _get_program():
    global _PROGRAM
    if _PROGRAM is None:
        _PROGRAM = _build_program()
    return _PROGRAM


def _split_fp8(a):
    """Split float32 array into fp8e4m3 hi + lo parts (a ~ hi + lo)."""
    e4 = ml_dtypes.float8_e4m3fn
    hi = a.astype(e4)
    lo = (a - hi.astype(np.float32)).astype(e4)
    return hi, lo


def _prep_inputs(x, Wq, bq, Wk, bk, Wv, bv, Wo, bo):
    """Build the 8 per-core input maps (host-side sharding + fp8 split)."""
    bf = ml_dtypes.bfloat16
    x = np.asarray(x, dtype=np.float32)
    WqT = np.ascontiguousarray(np.asarray(Wq, np.float32).T) * WSCALE
    WkT = np.ascontiguousarray(np.asarray(Wk, np.float32).T) * WSCALE
    WvT = np.ascontiguousarray(np.asarray(Wv, np.float32).T) * WSCALE
    WoT = np.ascontiguousarray(np.asarray(Wo, np.float32).T)  # [D, D] (f, d)

    # per-batch x^T fp8 splits (shared by the 2 head-group cores)
    xsplits = []
    for b in range(B):
        xT = np.ascontiguousarray(x[b].T)
        hi, lo = _split_fp8(xT)
        xsplits.append((hi.reshape(DC, P, T), lo.reshape(DC, P, T)))

    in_maps = []
    for c in range(NCORES):
        b, g = divmod(c, GROUPS)
        fsl = slice(g * F, (g + 1) * F)
        m = {"xh": xsplits[b][0], "xl": xsplits[b][1]}
        for name, WT in (("q", WqT), ("k", WkT), ("v", WvT)):
            hi, lo = _split_fp8(np.ascontiguousarray(WT[:, fsl]))
            m[f"w{name}h"] = hi.reshape(DC, P, F)
            m[f"w{name}l"] = lo.reshape(DC, P, F)
        m["wo"] = np.ascontiguousarray(
            WoT[fsl, :]).astype(bf).reshape(HG, P, D)
        m["bq"] = np.ascontiguousarray(
            np.asarray(bq, np.float32)[fsl].reshape(HG, P).T)
        m["bk"] = np.ascontiguousarray(
            np.asarray(bk, np.float32)[fsl].reshape(HG, P).T)
        m["bv"] = np.ascontiguousarray(
            np.asarray(bv, np.float32)[fsl].reshape(HG, P).T)
        in_maps.append(m)
    return in_maps


def _combine(results, bo):
    bo = np.asarray(bo, np.float32)
    out = np.empty((B, T, D), dtype=np.float32)
    for b in range(B):
        oT = (results[b * GROUPS]["out"].reshape(D, T).astype(np.float32)
              + results[b * GROUPS + 1]["out"].reshape(D, T).astype(
                  np.float32))
        out[b] = oT.T + bo[None, :]
    return out


def kernel(x, Wq, bq, Wk, bk, Wv, bv, Wo, bo):
    from concourse.bass_utils import run_bass_kernel_spmd

    nc = _get_program()
    in_maps = _prep_inputs(x, Wq, bq, Wk, bk, Wv, bv, Wo, bo)
    res = run_bass_kernel_spmd(nc, in_maps, list(range(NCORES))).results
    return _combine(res, bo)


# revision 4
# speedup vs baseline: 1.2366x; 1.2366x over previous
"""Trainium2 Bass kernel: multi-head attention (B=4, T=2048, D=2048, H=16).

Sharding: 8 cores = 4 batches x 2 head-groups (tensor-parallel heads, data-
parallel batch). Each core handles one batch and 8 heads (f-slice of 1024
columns of the QKV projections / rows of the out-projection). Host sums the
two partial out-projection results per batch and adds the output bias.

Per-core pipeline (all matmuls bf16 inputs, fp32 PSUM accumulation):
  Phase A: qT[f,t], kT[f,t] (weight-stationary), v[t,f] (x-stationary)
           from xT[d,t] streamed in t-blocks of 512; weights live in a
           17-slot chunk pool so the next pass's weight DMA overlaps the
           current pass's matmuls.
  Phase B: per (head, q-half) iteration i: S^T[k,q] = kT.T @ qT on PE;
           exp(scale*S^T) on ScalarE -> P^T bf16; PV with ones-augmented V
           gives [q, dh | rowsum]; normalize rows (DVE reciprocal + scale),
           PE-transpose y -> yT[f,t] (+bv fused, softmax rows sum to 1).
           Iterations are software-pipelined: the PV/normalize work of
           iteration i-1 is interleaved between the 16 score chunks of
           iteration i so PE and ScalarE both stay busy (B is paced by
           ScalarE's exp throughput).
  Phase C: out^T[d,t] = WoT.T @ yT accumulated over f-chunks, written bf16;
           host transposes back, sums the two head-group partials in f32
           and adds the output bias.  Wo tiles are prefetched during B.
"""

import sys

if "/opt/trn_rl_repo" not in sys.path:
    sys.path.insert(0, "/opt/trn_rl_repo")

import numpy as np
import ml_dtypes

D = 2048          # d_model
T = 2048          # sequence length
B = 4             # batch
H = 16            # total heads
DH = 128          # head dim
GROUPS = 2        # head groups (tensor-parallel factor per batch)
HG = H // GROUPS  # heads per core = 8
F = HG * DH       # per-core projection width = 1024
P = 128
DC = D // P       # 16 contraction chunks
TC = T // P       # 16 t chunks
NCORES = 8
SCALE = float(1.0 / np.sqrt(DH))

_PROGRAM = None


def _build_program():
    import concourse.bass as bass
    import concourse.tile as tile
    from concourse import bacc, mybir
    from concourse.bass import ts, ds
    from concourse.masks import make_identity

    bf16 = mybir.dt.bfloat16
    f32 = mybir.dt.float32

    nc = bacc.Bacc("TRN2", target_bir_lowering=False, debug=False,
                   num_devices=NCORES)

    xT_d = nc.dram_tensor("xT", [DC, P, T], bf16, kind="ExternalInput")
    wq_d = nc.dram_tensor("wq", [DC, P, F], bf16, kind="ExternalInput")
    wk_d = nc.dram_tensor("wk", [DC, P, F], bf16, kind="ExternalInput")
    wv_d = nc.dram_tensor("wv", [DC, P, F], bf16, kind="ExternalInput")
    wo_d = nc.dram_tensor("wo", [HG, P, D], bf16, kind="ExternalInput")
    bq_d = nc.dram_tensor("bq", [P, HG], f32, kind="ExternalInput")
    bk_d = nc.dram_tensor("bk", [P, HG], f32, kind="ExternalInput")
    bv_d = nc.dram_tensor("bv", [P, HG], f32, kind="ExternalInput")
    out_d = nc.dram_tensor("out", [DC, P, T], bf16, kind="ExternalOutput")

    Exp = mybir.ActivationFunctionType.Exp
    Identity = mybir.ActivationFunctionType.Identity

    with tile.TileContext(nc) as tc:
        from contextlib import ExitStack
        with ExitStack() as ctx:
            # ---- persistent pools (allocated first, live whole kernel) ----
            const = ctx.enter_context(tc.tile_pool(name="const", bufs=1))
            qkt = ctx.enter_context(tc.tile_pool(name="qkt", bufs=1))
            vpool = ctx.enter_context(tc.tile_pool(name="vpool", bufs=1))
            ytp = ctx.enter_context(tc.tile_pool(name="ytp", bufs=1))

            ident = const.tile([P, P], bf16, tag="ident")
            make_identity(nc, ident)
            zero_b = const.tile([P, 1], f32, tag="zerob")
            nc.vector.memset(zero_b[:], 0.0)
            bq_sb = const.tile([P, HG], f32, tag="bq")
            bk_sb = const.tile([P, HG], f32, tag="bk")
            bv_sb = const.tile([P, HG], f32, tag="bv")
            nc.sync.dma_start(bq_sb[:], bq_d[:])
            nc.sync.dma_start(bk_sb[:], bk_d[:])
            nc.sync.dma_start(bv_sb[:], bv_d[:])

            qT = [qkt.tile([P, T], bf16, tag=f"qT{h}", name=f"qT{h}")
                  for h in range(HG)]
            kT = [qkt.tile([P, T], bf16, tag=f"kT{h}", name=f"kT{h}")
                  for h in range(HG)]
            v_sb = vpool.tile([P, TC, HG, DH + 1], bf16, tag="v")
            yT = ytp.tile([P, HG, T], bf16, tag="yT")

            # force early allocation of persistent pools (first-use order)
            nc.vector.memset(qT[0][:, 0:1], 0.0)
            nc.vector.memset(v_sb[:, :, :, DH:DH + 1], 1.0)
            nc.vector.memset(yT[:, 0, 0:1], 0.0)

            # ---------------- Phase A: projections ----------------
            with tc.tile_pool(name="wpool", bufs=17) as wpool, \
                 tc.tile_pool(name="xpool", bufs=2) as xpool, \
                 tc.tile_pool(name="ps_proj", bufs=4, space="PSUM") as ps_proj:
                for wd, bias_sb, kind in ((wq_d, bq_sb, "q"),
                                          (wk_d, bk_sb, "k"),
                                          (wv_d, None, "v")):
                    w_sb = []
                    for pc in range(DC):
                        wt = wpool.tile([P, F], bf16, tag="wch",
                                        name=f"w_{kind}{pc}")
                        nc.sync.dma_start(wt[:], wd[pc])
                        w_sb.append(wt)
                    for tcb in range(4):  # t-blocks of 512
                        xblk = xpool.tile([P, DC, 512], bf16, tag="xblk",
                                          name=f"xblk_{kind}{tcb}")
                        src = xT_d[:, :, ds(tcb * 512, 512)].rearrange(
                            "c p t -> p c t")
                        for dg in range(DC):
                            nc.sync.dma_start(xblk[:, ds(dg, 1)],
                                              src[:, ds(dg, 1)])
                        if kind != "v":
                            dst = qT if kind == "q" else kT
                            for h in range(HG):
                                ps = ps_proj.tile([P, 512], f32, tag="ps512",
                                                  name=f"ps_{kind}{tcb}{h}")
                                for dc in range(DC):
                                    nc.tensor.matmul(
                                        ps[:],
                                        w_sb[dc][:, ds(h * DH, DH)],
                                        xblk[:, dc],
                                        start=(dc == 0), stop=(dc == DC - 1))
                                nc.scalar.activation(
                                    dst[h][:, ds(tcb * 512, 512)], ps[:],
                                    Identity, bias=bias_sb[:, ds(h, 1)])
                        else:
                            for tsub in range(4):
                                tc_ = tcb * 4 + tsub
                                psl = ps_proj.tile([P, 512], f32, tag="ps512",
                                                   name=f"psl{tc_}")
                                psr = ps_proj.tile([P, 512], f32, tag="ps512",
                                                   name=f"psr{tc_}")
                                for dc in range(DC):
                                    lhs = xblk[:, dc, ds(tsub * P, P)]
                                    nc.tensor.matmul(
                                        psl[:], lhs, w_sb[dc][:, 0:512],
                                        start=(dc == 0), stop=(dc == DC - 1))
                                    nc.tensor.matmul(
                                        psr[:], lhs, w_sb[dc][:, 512:1024],
                                        start=(dc == 0), stop=(dc == DC - 1))
                                nc.vector.tensor_copy(
                                    v_sb[:, tc_, 0:4, 0:DH],
                                    psl[:].rearrange("p (h d) -> p h d",
                                                     d=DH))
                                nc.vector.tensor_copy(
                                    v_sb[:, tc_, 4:8, 0:DH],
                                    psr[:].rearrange("p (h d) -> p h d",
                                                     d=DH))

            # -------- Phases B+C share the wop/osb SBUF pools -------------
            with tc.tile_pool(name="wop", bufs=4) as wop, \
                 tc.tile_pool(name="osb", bufs=6) as osb:
                wo_tiles = []

                def wo_tile(dch):
                    wo_t = wop.tile([P, HG, P], bf16, tag="wo",
                                    name=f"wo{dch}")
                    nc.sync.dma_start(
                        wo_t[:],
                        wo_d[:, :, ds(dch * P, P)].rearrange(
                            "h p d -> p h d"))
                    return wo_t

                # ---------------- Phase B: attention ----------------
                with tc.tile_pool(name="ptpool", bufs=2) as ptpool, \
                     tc.tile_pool(name="ystage", bufs=4) as ystage, \
                     tc.tile_pool(name="rspool", bufs=4) as rspool, \
                     tc.tile_pool(name="ps_st", bufs=2,
                                  space="PSUM") as ps_st, \
                     tc.tile_pool(name="ps_pv", bufs=2,
                                  space="PSUM") as ps_pv, \
                     tc.tile_pool(name="ps_tr", bufs=2,
                                  space="PSUM") as ps_tr:
                    # prefetch first wo chunks during B (DMA idle here)
                    for dch in range(4):
                        wo_tiles.append(wo_tile(dch))

                    flat = [(h, half) for h in range(HG) for half in range(2)]
                    pt_prev = None
                    prv = None
                    for idx in range(len(flat) + 1):
                        cur = flat[idx] if idx < len(flat) else None
                        if cur is not None:
                            pt_cur = ptpool.tile([P, TC, T // 2], bf16,
                                                 tag="pt", name=f"pt{idx}")
                        pv = None
                        for kc in range(TC):
                            if cur is not None:
                                h, half = cur
                                q0 = half * (T // 2)
                                st = ps_st.tile([P, T // 2], f32, tag="st",
                                                name=f"st{idx}_{kc}")
                                for qc in range(2):
                                    nc.tensor.matmul(
                                        st[:, ds(qc * 512, 512)],
                                        kT[h][:, ds(kc * P, P)],
                                        qT[h][:, ds(q0 + qc * 512, 512)],
                                        start=True, stop=True)
                                nc.scalar.activation(pt_cur[:, kc], st[:],
                                                     Exp, bias=zero_b[:, :],
                                                     scale=SCALE)
                            if prv is not None:
                                ph, phalf = prv
                                pq0 = phalf * (T // 2)
                                qs, sub = divmod(kc, 2)
                                if sub == 0:
                                    pv = ps_pv.tile([P, DH + 1], f32,
                                                    tag="pv",
                                                    name=f"pv{idx}_{qs}")
                                for k2 in range(sub * 8, sub * 8 + 8):
                                    nc.tensor.matmul(
                                        pv[:],
                                        pt_prev[:, k2, ds(qs * P, P)],
                                        v_sb[:, k2, ph],
                                        start=(k2 == 0), stop=(k2 == TC - 1))
                                if sub == 1:
                                    rs = rspool.tile([P, 1], f32, tag="rs",
                                                     name=f"rs{idx}_{qs}")
                                    nc.vector.reciprocal(rs[:],
                                                         pv[:, DH:DH + 1])
                                    yst = ystage.tile([P, P], bf16,
                                                      tag="yst",
                                                      name=f"yst{idx}_{qs}")
                                    nc.vector.tensor_scalar_mul(
                                        yst[:], pv[:, 0:DH], rs[:])
                                    tr = ps_tr.tile([P, P], bf16, tag="tr",
                                                    name=f"tr{idx}_{qs}")
                                    nc.tensor.transpose(tr[:], yst[:],
                                                        ident[:])
                                    nc.vector.tensor_scalar_add(
                                        yT[:, ph, ds(pq0 + qs * P, P)],
                                        tr[:], bv_sb[:, ds(ph, 1)])
                        pt_prev = pt_cur
                        prv = cur

                # ---------------- Phase C: out-projection ----------------
                with tc.tile_pool(name="ps_o", bufs=8, space="PSUM") as ps_o:
                    for dch in range(DC):
                        wo_t = wo_tiles[dch] if dch < 4 else wo_tile(dch)
                        pso = [ps_o.tile([P, 512], f32, tag="pso",
                                         name=f"pso{dch}_{i}")
                               for i in range(4)]
                        for fc in range(HG):
                            for tcb in range(4):
                                nc.tensor.matmul(
                                    pso[tcb][:],
                                    wo_t[:, fc],
                                    yT[:, fc, ds(tcb * 512, 512)],
                                    start=(fc == 0), stop=(fc == HG - 1))
                        for tcb in range(4):
                            ot = osb.tile([P, 512], bf16, tag="ot",
                                          name=f"ot{dch}_{tcb}")
                            nc.vector.tensor_copy(ot[:], pso[tcb][:])
                            nc.sync.dma_start(
                                out_d[dch, :, ds(tcb * 512, 512)], ot[:])

    nc.compile()
    return nc


def _get_program():
    global _PROGRAM
    if _PROGRAM is None:
        _PROGRAM = _build_program()
    return _PROGRAM


def _prep_inputs(x, Wq, bq, Wk, bk, Wv, bv, Wo, bo):
    """Build the 8 per-core input maps (host-side sharding, free)."""
    bf = ml_dtypes.bfloat16
    x = np.asarray(x, dtype=np.float32)
    WqT = np.ascontiguousarray(np.asarray(Wq, np.float32).T)  # [D, D]
    WkT = np.ascontiguousarray(np.asarray(Wk, np.float32).T)
    WvT = np.ascontiguousarray(np.asarray(Wv, np.float32).T)
    WoT = np.ascontiguousarray(np.asarray(Wo, np.float32).T)  # [D, D] (f, d)

    in_maps = []
    for c in range(NCORES):
        b, g = divmod(c, GROUPS)
        fsl = slice(g * F, (g + 1) * F)
        xT = np.ascontiguousarray(x[b].T).astype(bf).reshape(DC, P, T)
        m = {
            "xT": xT,
            "wq": np.ascontiguousarray(WqT[:, fsl]).astype(bf).reshape(
                DC, P, F),
            "wk": np.ascontiguousarray(WkT[:, fsl]).astype(bf).reshape(
                DC, P, F),
            "wv": np.ascontiguousarray(WvT[:, fsl]).astype(bf).reshape(
                DC, P, F),
            "wo": np.ascontiguousarray(WoT[fsl, :]).astype(bf).reshape(
                HG, P, D),
            "bq": np.ascontiguousarray(
                np.asarray(bq, np.float32)[fsl].reshape(HG, P).T),
            "bk": np.ascontiguousarray(
                np.asarray(bk, np.float32)[fsl].reshape(HG, P).T),
            "bv": np.ascontiguousarray(
                np.asarray(bv, np.float32)[fsl].reshape(HG, P).T),
        }
        in_maps.append(m)
    return in_maps


def _combine(results, bo):
    bo = np.asarray(bo, np.float32)
    out = np.empty((B, T, D), dtype=np.float32)
    for b in range(B):
        oT = (results[b * GROUPS]["out"].reshape(D, T).astype(np.float32)
              + results[b * GROUPS + 1]["out"].reshape(D, T).astype(
                  np.float32))
        out[b] = oT.T + bo[None, :]
    return out


def kernel(x, Wq, bq, Wk, bk, Wv, bv, Wo, bo):
    from concourse.bass_utils import run_bass_kernel_spmd

    nc = _get_program()
    in_maps = _prep_inputs(x, Wq, bq, Wk, bk, Wv, bv, Wo, bo)
    res = run_bass_kernel_spmd(nc, in_maps, list(range(NCORES))).results
    return _combine(res, bo)
